# revision 21
# baseline (speedup 1.0000x reference)
"""CrossEntropyBoundSmoothLoss on 8 Trainium2 NeuronCores (Bass/Tile).

Math: loss*N = sum_t [ Tt_t * ln(Z_t) - sum_l T[t,l]*X[t,l] ],
Z_t = sum_l exp(X[t,l]), T = smoothed targets.

Device-side structure (per core, 16384 rows x 200 labels):
  - Z is label-permutation-invariant, so the 200 columns are re-packed
    into three contiguous engine sets [0:A | A:A+P | A+P:200] and the
    exp work is SPLIT 3 ways (ACT alone would be 25600 els/partition
    @ 1/cyc/lane = 21.3us):
      * ACT: exp of A labels/row from fp8 e3m4 input (LUT exp,
        1 el/cycle, ~0.83 ns/el).
      * Pool (gpsimd) + DVE: the rest from bf16 input via the
        Schraudolph bit trick: i16 = rint(x*128/ln2 + 16256 - 7.25);
        the int16 bits, bitcast to bf16, ARE exp(x) to ~1.8% sawtooth
        (mean bias ~1e-3, calibrated; the loss gate is 2e-2). Pool runs
        a software tensor_scalar (slower than the cost model's guess;
        P is tuned on HW), DVE a HW tensor_scalar in 4x mode.
  - One whole-body et buffer [128, 128, 200] so the row-sum tree is
    5 big DVE instructions per body: pairwise bf16 tensor_tensor tree
    in 2x mode (200->100->50->25, then in-place 25->13) + one 1x
    tensor_reduce over the last 13.
  - DMA: xtb goes FIRST on the sync FIFO queue each chunk (ahead of the
    bigger x8a) so Schraudolph inputs are not the late link; issuing
    them from Pool would delay them behind Pool's own TS work.
  - The loop body is unrolled U times inside For_i: the back-edge costs
    ~5us even with staggered_reset; 1/U amortizes it.
  - The dot sum_l T*X: T has <=5 nonzeros/row; host gathers X at those
    columns and ships xg bf16 [rows,5] + deduped int8 weights wg; the
    device reduces them with one affine_mul_reduce.
  - Tail: Ln(Z) on ACT (exp+ln share the natural_log_exp table set),
    AMR tt*logZ, out [128,4] per core; host sums partials / N.

Sharding: whole sequences per core (rows row-major), host combines.
"""

import numpy as np

B = 64
S = 2048
L = 200
E = 0.1
D = 2
N_ROWS = B * S            # 131072
N_CORES = 8
RPC = N_ROWS // N_CORES   # 16384 rows per core
ROWS_PP = RPC // 128      # 128 rows per partition
NGATH = 5                 # candidate dot columns per row
TILE_PLAN = (64, 64)      # DMA/ACT chunk sizes (slabs); sums to ROWS_PP
BUFS = 2
BOUND_IDS = np.arange(0, L, 10)

# exp split: ACT takes labels [0, A_TOT) from fp8; the rest ships bf16
# and goes through Schraudolph: [A_TOT, A_TOT+P_TOT) on Pool (gpsimd),
# [A_TOT+P_TOT, 200) on DVE.
A_TOT = 138
P_TOT = 42
# Schraudolph constants (bf16 target): i16 = x*SCH_S + SCH_B
SCH_S = 128.0 / float(np.log(2.0))   # 184.6649652
SCH_B = 16256.0 - 7.25               # 127*128 + calibrated bias


def build_targets_int8(label_ids: np.ndarray) -> np.ndarray:
    """Dense smoothed targets * 120 as int8, [N_ROWS, L]. Exact.

    Reproduces reference semantics: boundary occurrences at t' spread
    E/w over [t'-D, t'+D] (within the sequence) with 1-E at the center;
    overlapping windows of the same label resolve to the largest t'
    (ascending-t' scatter, last write wins). Non-boundary own labels get
    plain one-hot.
    """
    lab = label_ids.reshape(B, S).astype(np.int64)
    is_bound = np.zeros(L, bool)
    is_bound[BOUND_IDS] = True

    T = np.zeros((B, S, L), np.int8)
    t = np.arange(S)
    for o in range(-D, D + 1):  # ascending t' = t+o: last write wins
        tp = t + o
        valid = (tp >= 0) & (tp < S)
        tpc = np.clip(tp, 0, S - 1)
        cand_lab = lab[:, tpc]                       # [B, S]
        vmask = valid[None, :] & is_bound[cand_lab]  # [B, S]
        w = np.minimum(S - 1, tpc + D) - np.maximum(0, tpc - D)
        val = np.where(tp == t, 108, 12 // np.maximum(w, 1))  # {108,3,4,6}
        for b in range(B):
            m = vmask[b]
            T[b, t[m], cand_lab[b, m]] = val[m]
    nb = ~is_bound[lab]  # non-boundary own labels -> one-hot
    bidx, tidx = np.nonzero(nb)
    T[bidx, tidx, lab[bidx, tidx]] = 120
    return T.reshape(N_ROWS, L)


def build_gather(label_ids: np.ndarray, T8: np.ndarray):
    """(cols, wg): <=5 candidate columns per row + deduped int8 weights
    such that sum_j wg[t,j]*X[t,cols[t,j]] == sum_l T8[t,l]*X[t,l]."""
    lab = label_ids.reshape(B, S).astype(np.int64)
    t = np.arange(S)
    cols = np.zeros((B, S, NGATH), np.int64)
    for j, o in enumerate((-2, -1, 0, 1, 2)):
        tp = np.clip(t + o, 0, S - 1)
        c = lab[:, tp]
        invalid = (t + o < 0) | (t + o >= S)
        c[:, invalid] = lab[:, invalid]  # own label -> deduped below
        cols[:, :, j] = c
    cols = cols.reshape(N_ROWS, NGATH)
    dup = np.zeros((N_ROWS, NGATH), bool)
    for j in range(1, NGATH):
        for k in range(j):
            dup[:, j] |= cols[:, j] == cols[:, k]
    wg = np.take_along_axis(T8, cols, axis=1).astype(np.int16)
    wg[dup] = 0
    return cols, wg.astype(np.int8)


_NC_CACHE = {}


def _patch_act_tables(bacc_mod):
    """Restrict the activation-table chooser to the set that has BOTH Exp
    and Ln, so the kernel loads one table once instead of swapping
    exp_and_others <-> natural_log mid-kernel (~1.3us + an ACT stall)."""
    if getattr(bacc_mod, "_ant_tables_patched", False):
        return
    orig = bacc_mod.get_activation_tables

    def patched(arch):
        import concourse.mybir as mybir

        tables = orig(arch)
        if not any("natural_log_exp" in k for k in tables):
            return tables
        # keep list order/length (set ids index the act_info list); just
        # make natural_log_exp_and_others the only set offering Exp/Ln
        strip = {
            mybir.ActivationFunctionType.Exp,
            mybir.ActivationFunctionType.Ln,
        }
        return {
            k: (v if "natural_log_exp" in k else set(v) - strip)
            for k, v in tables.items()
        }

    bacc_mod.get_activation_tables = patched
    bacc_mod._ant_tables_patched = True


def _build_nc(tile_plan=TILE_PLAN, bufs: int = BUFS, loop_n: int = 1,
              staggered: bool = True, unroll: int = 16,
              a: int = A_TOT, p: int = P_TOT):
    v = L - a - p                      # DVE Schraudolph labels
    nb = L - a                         # bf16 labels (Pool + DVE)
    key = (tuple(tile_plan), bufs, loop_n, staggered, unroll, a, p)
    if key in _NC_CACHE:
        return _NC_CACHE[key]
    assert sum(tile_plan) == ROWS_PP
    from contextlib import ExitStack

    import concourse.bacc as bacc
    import concourse.mybir as mybir
    import concourse.tile as tile

    _patch_act_tables(bacc)

    f32 = mybir.dt.float32
    bf16 = mybir.dt.bfloat16
    i16 = mybir.dt.int16
    f8 = mybir.dt.float8e3
    nc = bacc.Bacc("TRN2", debug=False, num_devices=N_CORES)
    # x8a: fp8 ACT share = packed labels [0, a)
    x8a_d = nc.dram_tensor("x8a", [RPC, a], f8, kind="ExternalInput")
    # xb: bf16 Schraudolph share = packed labels [a, 200)
    xb_d = nc.dram_tensor("xb", [RPC, nb], bf16, kind="ExternalInput")
    xg_d = nc.dram_tensor("xg", [128, ROWS_PP * NGATH], bf16, kind="ExternalInput")
    wg_d = nc.dram_tensor("wg", [128, ROWS_PP * NGATH], mybir.dt.int8,
                          kind="ExternalInput")
    tt_d = nc.dram_tensor("tt", [128, ROWS_PP], f32, kind="ExternalInput")
    out_d = nc.dram_tensor("out", [128, 4], f32, kind="ExternalOutput")

    with tile.TileContext(nc) as tc, ExitStack() as ctx:
        xp8 = ctx.enter_context(tc.tile_pool(name="xp8", bufs=bufs + 1))
        xpb = ctx.enter_context(tc.tile_pool(name="xpb", bufs=bufs + 1))
        ep = ctx.enter_context(tc.tile_pool(name="ep", bufs=bufs))
        # o-pools are written and read only by the in-order DVE: bufs=1
        o1p = ctx.enter_context(tc.tile_pool(name="o1p", bufs=1))
        o2p = ctx.enter_context(tc.tile_pool(name="o2p", bufs=1))
        o3p = ctx.enter_context(tc.tile_pool(name="o3p", bufs=1))
        sp = ctx.enter_context(tc.tile_pool(name="sp", bufs=1))

        z_all = sp.tile([128, ROWS_PP], f32)
        logz = sp.tile([128, ROWS_PP], f32)
        lgz_dst = sp.tile([128, ROWS_PP], f32)
        tt_sb = sp.tile([128, ROWS_PP], f32)
        xg_sb = sp.tile([128, ROWS_PP * NGATH], bf16)
        wg_sb = sp.tile([128, ROWS_PP * NGATH], mybir.dt.int8)
        dot_dst = sp.tile([128, ROWS_PP * NGATH], f32)
        out_sb = sp.tile([128, 4], f32)

        # side inputs go via SWDGE (gpsimd/Pool) so they reach SBUF in
        # parallel with the logits tiles on the HWDGE rings
        nc.gpsimd.dma_start(tt_sb[:], tt_d.ap())
        nc.gpsimd.dma_start(xg_sb[:], xg_d.ap())
        nc.gpsimd.dma_start(wg_sb[:], wg_d.ap())

        # dot term: one AMR over the gathered [rows,5] pairs
        nc.vector.affine_mul_reduce(
            out=dot_dst[:],
            accum_out=out_sb[:, 2:3],
            in0=wg_sb[:],
            in1=xg_sb[:],
            scale=1.0 / 120.0,
            bias=0.0,
        )
        nc.vector.memset(out_sb[:, 3:4], 0.0)

        import contextlib

        # Unroll U bodies per For_i iteration: the loop back-edge has an
        # all-engine sync cost (~several us even staggered), amortized 1/U.
        U = unroll if loop_n > 1 and loop_n % unroll == 0 else 1
        loop_cm = (
            tc.For_i(0, loop_n // U, 1, staggered_reset=staggered)
            if loop_n // U > 1
            else contextlib.nullcontext()
        )
        with loop_cm:
          for _u in range(U):
            et = ep.tile([128, ROWS_PP, L], bf16)
            off = 0
            for rp in tile_plan:
                # xtb first on the FIFO sync queue: the Schraudolph inputs
                # must not queue behind the bigger x8a transfer (and Pool
                # must not issue them — its own TS work would delay them)
                rowsb = xb_d.ap()[off * 128 : (off + rp) * 128, :]
                xbv = rowsb.rearrange("(p s) l -> p s l", p=128, s=rp)
                xtb = xpb.tile([128, rp, nb], bf16)
                nc.sync.dma_start(xtb[:], xbv)

                rows8 = x8a_d.ap()[off * 128 : (off + rp) * 128, :]
                x8v = rows8.rearrange("(p s) l -> p s l", p=128, s=rp)
                xt8 = xp8.tile([128, rp, a], f8)
                nc.sync.dma_start(xt8[:], x8v)

                nc.scalar.activation(
                    et[:, off : off + rp, 0:a], xt8[:],
                    mybir.ActivationFunctionType.Exp,
                )
                # Schraudolph exp: int16(x*S+B) bits = bf16 exp(x)
                if p:
                    nc.gpsimd.tensor_scalar(
                        et[:, off : off + rp, a : a + p].bitcast(i16),
                        xtb[:, :, 0:p],
                        SCH_S,
                        SCH_B,
                        mybir.AluOpType.mult,
                        mybir.AluOpType.add,
                    )
                if v:
                    nc.vector.tensor_scalar(
                        et[:, off : off + rp, a + p : L].bitcast(i16),
                        xtb[:, :, p:nb],
                        SCH_S,
                        SCH_B,
                        mybir.AluOpType.mult,
                        mybir.AluOpType.add,
                    )
                off += rp

            # whole-body row-sum tree on DVE (bf16 2x tensor_tensor)
            o1 = o1p.tile([128, ROWS_PP, 100], bf16)
            nc.vector.tensor_add(o1[:], et[:, :, 0:100], et[:, :, 100:200])
            o2 = o2p.tile([128, ROWS_PP, 50], bf16)
            nc.vector.tensor_add(o2[:], o1[:, :, 0:50], o1[:, :, 50:100])
            o3 = o3p.tile([128, ROWS_PP, 25], bf16)
            nc.vector.tensor_add(o3[:], o2[:, :, 0:25], o2[:, :, 25:50])
            # level 4 in place: o3[0:12] += o3[13:25] (el 12 untouched), so
            # the final 1x reduce runs over 13 contiguous els, not 25
            nc.vector.tensor_add(
                o3[:, :, 0:12], o3[:, :, 0:12], o3[:, :, 13:25]
            )
            nc.vector.tensor_reduce(
                z_all[:],
                o3[:, :, 0:13],
                axis=mybir.AxisListType.X,
                op=mybir.AluOpType.add,
            )

        # logZ term: Ln on ACT, then tt*logZ via AMR
        nc.scalar.activation(
            logz[:], z_all[:], mybir.ActivationFunctionType.Ln,
        )
        nc.vector.affine_mul_reduce(
            out=lgz_dst[:],
            accum_out=out_sb[:, 0:1],
            in0=tt_sb[:],
            in1=logz[:],
            scale=1.0,
            bias=0.0,
        )
        nc.vector.memset(out_sb[:, 1:2], 0.0)
        nc.sync.dma_start(out_d.ap(), out_sb[:])

    nc.compile()
    _NC_CACHE[key] = nc
    return nc


def make_in_maps(logits: np.ndarray, label_ids: np.ndarray,
                 tile_plan=TILE_PLAN, a: int = A_TOT):
    import ml_dtypes

    X = np.ascontiguousarray(np.asarray(logits, dtype=np.float32))
    lab = np.asarray(label_ids).astype(np.int64)
    T8 = build_targets_int8(lab)
    cols, wg = build_gather(lab, T8)
    xg = np.take_along_axis(X, cols, axis=1).astype(ml_dtypes.bfloat16)
    Tt = (wg.astype(np.int64).sum(axis=1) / 120.0).astype(np.float32)
    # ACT share (fp8): labels [0:a); Schraudolph share (bf16): [a:200)
    x8a = np.ascontiguousarray(X[:, 0:a]).astype(ml_dtypes.float8_e3m4)
    xb = np.ascontiguousarray(X[:, a:L]).astype(ml_dtypes.bfloat16)

    in_maps = []
    for c in range(N_CORES):
        sl = slice(c * RPC, (c + 1) * RPC)
        # tt[p, off+s] = Tt[row off*128 + p*rp + s] per chunk
        tt_c = np.empty((128, ROWS_PP), np.float32)
        off = 0
        for rp in tile_plan:
            seg = Tt[sl][off * 128 : (off + rp) * 128].reshape(128, rp)
            tt_c[:, off : off + rp] = seg
            off += rp
        in_maps.append(
            {
                "x8a": x8a[sl],
                "xb": xb[sl],
                "xg": np.ascontiguousarray(
                    xg[sl].reshape(128, ROWS_PP * NGATH)
                ),
                "wg": np.ascontiguousarray(
                    wg[sl].reshape(128, ROWS_PP * NGATH)
                ),
                "tt": tt_c,
            }
        )
    return in_maps


def combine(results) -> np.ndarray:
    total = 0.0
    for r in results:
        o = r["out"].astype(np.float64)
        total += o[:, 0].sum() + o[:, 1].sum() - o[:, 2].sum()
    return np.asarray(np.float32(total / N_ROWS))


def kernel(logits, label_ids) -> np.ndarray:
    from concourse.bass_utils import run_bass_kernel_spmd

    nc = _build_nc()
    in_maps = make_in_maps(logits, label_ids)
    res = run_bass_kernel_spmd(nc, in_maps, core_ids=list(range(N_CORES)))
    return combine(res.results)
